# revision 27
# baseline (speedup 1.0000x reference)
"""Trainium2 Bass kernel for nn_BBBLSTM: LayerNorm -> LSTM(25->128, T=30) -> MLP head.

Sharding: data-parallel, batch 8192 -> 1024 per core across 8 NeuronCores.
Weights replicated. No collectives.

v2 design (engine-balanced, latency-pipelined):
  - Phase 0 (batch-major): x staged as [BC, 32t, 32f] bf16 (zero-padded).
    LN stats (sum / sumsq reduce, Square on Act), rstd via sqrt+reciprocal.
    Apply xs = x*rstd in batch-major (free covers all 30 timesteps), the
    -mu*rstd term rides an extra "moving row" (col 26) against a
    column-sum stationary row (rank-1 folding); col 25 = ones for bias.
    Result written to xnd [BC, 1024] DRAM.
  - DMA-transpose (xbar, 16x128 tiles) turns xnd into feature-major
    xt4[g] = [128p = 4t x 32f, 1024 batch] tiles, one per 4-timestep group.
  - Recurrence, K=4 batch chunks of 256 (pipelined chains):
    8 matmuls/chunk -> psum [128, 4*256] (gates i|f|g2|o, g pre-doubled),
    one Sigmoid per chunk covers all four gates (tanh g = 2*sig(2g)-1).
    tanh(c): chunks 0,1 on Act (merged instr), chunks 2,3 as deg-7
    odd polynomial on DVE.  fc = sig_f*c and ig (chunks 0,1) on GPSIMD.
  - MLP head on h_last, output [2, BC] -> host transposes.
"""

import ml_dtypes
import numpy as np

BF16 = ml_dtypes.bfloat16

import concourse.bacc as bacc
import concourse.bass as bass
import concourse.mybir as mybir
from concourse.tile import TileContext

B, T, F, H = 8192, 30, 25, 128
T2 = 32                   # padded timestep slots
FP_ = 32                  # padded feature slots (25 feat + ones + mur + pad)
NCORES = 8
BC = B // NCORES          # 1024 batch rows per core
G = 4 * H                 # 512 gate width
NB = 256                  # batch chunk (4 chunks)
NCHUNK = BC // NB
EPS = 1e-5
FP = mybir.dt.float32
BF = mybir.dt.bfloat16
AF = mybir.ActivationFunctionType
OP = mybir.AluOpType
AX = mybir.AxisListType

# deg-7 odd minimax fit of tanh on [-1.95, 1.95] (maxerr 2.2e-3)
TA0, TA1, TA2, TA3 = 0.9884014, -0.28059538, 0.06143643, -0.00579145

_CACHE = {}

# engine/config knobs (overridable before _build_nc for tuning)
CFG = {
    # per-chunk engine for each elementwise op ("vector" = DVE, "gpsimd" = Pool)
    "tg": ("gpsimd",) * 4,
    "ig": ("gpsimd",) * 4,
    "fc": ("gpsimd",) * 4,
    "add": ("gpsimd",) * 4,
    "h": ("vector",) * 4,
    "poly": ("vector",) * 4,
    "act_tanh_chunks": (0, 1),   # chunks using Act tanh (merged if contiguous)
    "apply_eng": "gpsimd",       # phase-0 LN apply
    "sig_bufs": 3,
    "tmp_bufs": 3,
}


def _bcast_f(ap2d, n):
    """[P, K] AP -> [P, K, n] with stride-0 inner broadcast."""
    return bass.AP(tensor=ap2d.tensor, offset=ap2d.offset,
                   ap=[ap2d.ap[0], ap2d.ap[1], [0, n]])


def _build_nc():
    nc = bacc.Bacc()

    x0p = nc.declare_dram_parameter("x0p", [BC, T2 * FP_], BF, isOutput=False)
    wa_rep = nc.declare_dram_parameter("wa_rep", [128, G], BF, isOutput=False)
    w_hh = nc.declare_dram_parameter("w_hh", [H, G], BF, isOutput=False)
    w1 = nc.declare_dram_parameter("w1", [H, H], BF, isOutput=False)
    b1 = nc.declare_dram_parameter("b1", [H, 1], FP, isOutput=False)
    w2 = nc.declare_dram_parameter("w2", [H, H // 2], BF, isOutput=False)
    b2 = nc.declare_dram_parameter("b2", [H // 2, 1], FP, isOutput=False)
    w3 = nc.declare_dram_parameter("w3", [H // 2, 2], BF, isOutput=False)
    b3 = nc.declare_dram_parameter("b3", [2, 1], FP, isOutput=False)
    out = nc.declare_dram_parameter("out", [2, BC], FP, isOutput=True)

    xnd = nc.dram_tensor("xnd", [BC, T2 * FP_], BF)

    from contextlib import ExitStack

    with TileContext(nc) as tc, ExitStack() as ctx:
        consts = ctx.enter_context(tc.tile_pool(name="consts", bufs=1))
        state = ctx.enter_context(tc.tile_pool(name="state", bufs=1))
        p0 = ctx.enter_context(tc.tile_pool(name="p0", bufs=3))
        p0s = ctx.enter_context(tc.tile_pool(name="p0s", bufs=6))
        sigp = ctx.enter_context(tc.tile_pool(name="sigp", bufs=CFG["sig_bufs"]))
        tmpp = ctx.enter_context(tc.tile_pool(name="tmpp", bufs=CFG["tmp_bufs"]))
        mlpp = ctx.enter_context(tc.tile_pool(name="mlpp", bufs=2))

        # ---- constants ----
        wa_sb = consts.tile([128, G], BF)
        nc.sync.dma_start(out=wa_sb, in_=wa_rep[:, :])
        whh_sb = consts.tile([H, G], BF)
        nc.gpsimd.dma_start(out=whh_sb, in_=w_hh[:, :])
        w1_sb = consts.tile([H, H], BF)
        nc.gpsimd.dma_start(out=w1_sb, in_=w1[:, :])
        b1_sb = consts.tile([H, 1], FP)
        nc.sync.dma_start(out=b1_sb, in_=b1[:, :])
        w2_sb = consts.tile([H, H // 2], BF)
        nc.gpsimd.dma_start(out=w2_sb, in_=w2[:, :])
        b2_sb = consts.tile([H // 2, 1], FP)
        nc.sync.dma_start(out=b2_sb, in_=b2[:, :])
        w3_sb = consts.tile([H // 2, 2], BF)
        nc.gpsimd.dma_start(out=w3_sb, in_=w3[:, :])
        b3_sb = consts.tile([2, 1], FP)
        nc.sync.dma_start(out=b3_sb, in_=b3[:, :])
        eps_sb = consts.tile([128, 1], FP)
        nc.vector.memset(eps_sb, EPS)

        # ---- phase 0: LN stats + apply, batch-major, 8 tiles of 128 rows ----
        NG = T2 // 4
        xnd_m = xnd[:, :].rearrange("b (g c) -> b g c", c=128)
        xt4 = [state.tile([128, BC], BF, name=f"xt4_{g}") for g in range(NG)]

        NT = BC // 128
        for i in range(NT):
            rows = slice(i * 128, (i + 1) * 128)
            x0 = p0.tile([128, T2 * FP_], BF, tag="x0")
            nc.gpsimd.dma_start(out=x0, in_=x0p[rows, :])
            x0v = x0[:, :].rearrange("p (t f) -> p t f", f=FP_)

            # stats reduces on unpadded [T2, F] strided views (free 800 not 1024)
            s1 = p0s.tile([128, T2], FP, tag="s1")
            nc.vector.tensor_reduce(out=s1, in_=x0v[:, :, 0:F], axis=AX.X,
                                    op=OP.add)
            x2 = p0.tile([128, T2 * FP_], BF, tag="x2")
            x2v = x2[:, :].rearrange("p (t f) -> p t f", f=FP_)
            nc.scalar.activation(x2v[:, :, 0:F], x0v[:, :, 0:F], AF.Square)
            s2 = p0s.tile([128, T2], FP, tag="s2")
            nc.vector.tensor_reduce(out=s2, in_=x2v[:, :, 0:F], axis=AX.X,
                                    op=OP.add)

            ve = nc.vector
            mu_n = p0s.tile([128, T2], BF, tag="mu_n")
            ve.tensor_scalar_mul(mu_n, s1, -1.0 / F)
            mu2 = p0s.tile([128, T2], FP, tag="mu2")
            ve.tensor_mul(mu2, mu_n, mu_n)
            mu2e = p0s.tile([128, T2], FP, tag="mu2e")
            ve.tensor_scalar_sub(mu2e, mu2, EPS)
            w0 = p0s.tile([128, T2], FP, tag="w0")
            ve.scalar_tensor_tensor(
                out=w0, in0=s2, scalar=1.0 / F, in1=mu2e,
                op0=OP.mult, op1=OP.subtract)
            if True:
                # Act Sqrt + exact reciprocal
                sd = p0s.tile([128, T2], FP, tag="sd")
                nc.scalar.activation(sd, w0, AF.Sqrt)
                r_ = p0s.tile([128, T2], BF, tag="r")
                with nc.allow_low_precision(reason="bf16 rstd for LN"):
                    nc.vector.reciprocal(r_, sd)
            elif False:
                # overlapped tiles: rsqrt = poly seed + 2 Newton iterations
                # on DVE so no act-table switch mid-recurrence
                RC0, RC1, RC2 = 2.36276903, -1.63315947, 0.37159572
                t0_ = p0s.tile([128, T2], FP, tag="t0")
                ve.tensor_scalar(out=t0_, in0=w0, scalar1=RC2, scalar2=RC1,
                                 op0=OP.mult, op1=OP.add)
                y0 = p0s.tile([128, T2], FP, tag="y0")
                ve.tensor_mul(y0, t0_, w0)
                yc = p0s.tile([128, T2], FP, tag="yc")
                ve.tensor_scalar_add(yc, y0, RC0)
                for it in range(2):
                    t1 = p0s.tile([128, T2], FP, tag=f"nt1_{it}")
                    ve.tensor_mul(t1, yc, yc)
                    t2 = p0s.tile([128, T2], FP, tag=f"nt2_{it}")
                    ve.tensor_mul(t2, w0, t1)
                    t3 = p0s.tile([128, T2], FP, tag=f"nt3_{it}")
                    ve.tensor_scalar(out=t3, in0=t2, scalar1=-0.5,
                                     scalar2=1.5, op0=OP.mult, op1=OP.add)
                    ycn = p0s.tile([128, T2], FP if it == 0 else BF,
                                   tag=f"ycn_{it}")
                    ve.tensor_mul(ycn, yc, t3)
                    yc = ycn
                r_ = yc

            xn = p0.tile([128, T2 * FP_], BF, tag="xn")
            xnv = xn[:, :].rearrange("p (t f) -> p t f", f=FP_)
            getattr(nc, CFG["apply_eng"]).tensor_mul(
                xnv, x0v, _bcast_f(r_[:, :], FP_))
            # col 26 = -mu*rstd (rank-1 row against column-sum stationary)
            nc.gpsimd.tensor_mul(xnv[:, :, 26:27], mu_n, r_)
            # col 25 = ones (bias row)
            nc.vector.memset(xnv[:, :, 25:26], 1.0)
            nc.sync.dma_start(out=xnd[rows, :], in_=xn)

            # transposes per batch-half: the left half (tiles 0-3) unblocks
            # chunks 0,1 of the recurrence while tiles 4-7 finish phase 0
            if i in (3, 7):
                half = i // 4
                hrows = slice(half * 512, (half + 1) * 512)
                for g in range(NG):
                    nc.sync.dma_start_transpose(out=xt4[g][:, hrows],
                                                in_=xnd_m[hrows, g, :])

        # ---- persistent LSTM state ----
        h = state.tile([H, BC], BF)
        c = state.tile([H, BC], BF)
        nc.vector.memset(h, 0.0)
        nc.vector.memset(c, 0.0)

        # gate column slices in psum: i | f | g2 | o
        SI, SF, SG, SO = (slice(k * NB, (k + 1) * NB) for k in range(4))

        ps_cm = tc.tile_pool(name="ps", bufs=1, space="PSUM")
        psp = ps_cm.__enter__()
        for t in range(T):
            g, rr = divmod(t, 4)
            pb = 32 * rr
            sigs = []
            for cc in range(NCHUNK):
                S = slice(cc * NB, (cc + 1) * NB)
                ps = psp.tile([128, 4 * NB], FP, tag=f"ps{cc}")
                for k, gsl in enumerate(
                        (slice(0, 128), slice(128, 256),
                         slice(256, 384), slice(384, 512))):
                    d = ps[:, k * NB:(k + 1) * NB]
                    nc.tensor.matmul(d, wa_sb[pb:pb + 32, gsl],
                                     xt4[g][pb:pb + 32, S],
                                     start=True, stop=False,
                                     tile_position=(pb, 0))
                    nc.tensor.matmul(d, whh_sb[:, gsl], h[:, S],
                                     start=False, stop=True)
                sig = sigp.tile([128, 4 * NB], BF, tag=f"sig{cc}")
                nc.scalar.activation(sig, ps, AF.Sigmoid)
                sigs.append(sig)

                E = lambda op: getattr(nc, CFG[op][cc])
                tg = tmpp.tile([128, NB], BF, tag=f"tg{cc}")
                E("tg").tensor_scalar(
                    out=tg, in0=sig[:, SG], scalar1=2.0, scalar2=-1.0,
                    op0=OP.mult, op1=OP.add)
                ig = tmpp.tile([128, NB], BF, tag=f"ig{cc}")
                E("ig").tensor_mul(ig, sig[:, SI], tg)
                fc = tmpp.tile([128, NB], BF, tag=f"fc{cc}")
                E("fc").tensor_mul(fc, sig[:, SF], c[:, S])
                E("add").tensor_add(c[:, S], fc, ig)

                if cc not in CFG["act_tanh_chunks"]:
                    # tanh(c) deg-7 odd polynomial
                    pe_ = getattr(nc, CFG["poly"][cc])
                    u = tmpp.tile([128, NB], BF, tag=f"u{cc}")
                    pe_.tensor_mul(u, c[:, S], c[:, S])
                    v = tmpp.tile([128, NB], BF, tag=f"v{cc}")
                    pe_.tensor_scalar(out=v, in0=u, scalar1=TA3,
                                      scalar2=TA2, op0=OP.mult, op1=OP.add)
                    w_ = tmpp.tile([128, NB], BF, tag=f"w{cc}")
                    pe_.tensor_mul(w_, v, u)
                    z = tmpp.tile([128, NB], BF, tag=f"z{cc}")
                    pe_.tensor_scalar_add(z, w_, TA1)
                    zz = tmpp.tile([128, NB], BF, tag=f"zz{cc}")
                    pe_.tensor_mul(zz, z, u)
                    p_ = tmpp.tile([128, NB], BF, tag=f"p{cc}")
                    pe_.tensor_scalar_add(p_, zz, TA0)
                    tc_ = tmpp.tile([128, NB], BF, tag=f"tc{cc}")
                    pe_.tensor_mul(tc_, p_, c[:, S])
                    E("h").tensor_mul(h[:, S], sigs[cc][:, SO], tc_)

            # Act-tanh chunks: exact tanh, merged instr if contiguous
            atc = CFG["act_tanh_chunks"]
            if atc:
                lo, hi = min(atc), max(atc) + 1
                tc01 = tmpp.tile([128, (hi - lo) * NB], BF, tag="tc01")
                nc.scalar.activation(tc01, c[:, lo * NB:hi * NB], AF.Tanh)
                for cc in atc:
                    S = slice(cc * NB, (cc + 1) * NB)
                    getattr(nc, CFG["h"][cc]).tensor_mul(
                        h[:, S], sigs[cc][:, SO],
                        tc01[:, (cc - lo) * NB:(cc - lo + 1) * NB])

        ps_cm.__exit__(None, None, None)

        # ---- MLP head ----
        ps2_cm = tc.tile_pool(name="ps2", bufs=2, space="PSUM")
        ps2 = ps2_cm.__enter__()
        for cc in range(2):
            S = slice(cc * 512, (cc + 1) * 512)
            m1 = ps2.tile([H, 512], FP, tag="m1")
            nc.tensor.matmul(m1, w1_sb, h[:, S], start=True, stop=True)
            y1 = mlpp.tile([H, 512], BF, tag="y1")
            nc.scalar.activation(y1, m1, AF.Relu, bias=b1_sb[:, 0:1])
            m2 = ps2.tile([H // 2, 512], FP, tag="m2")
            nc.tensor.matmul(m2, w2_sb, y1, start=True, stop=True)
            y2 = mlpp.tile([H // 2, 512], BF, tag="y2")
            nc.scalar.activation(y2, m2, AF.Relu, bias=b2_sb[:, 0:1])
            m3 = ps2.tile([2, 512], FP, tag="m3")
            nc.tensor.matmul(m3, w3_sb, y2, start=True, stop=True)
            y3 = mlpp.tile([2, 512], FP, tag="y3")
            nc.vector.tensor_scalar_add(y3, m3, b3_sb[:, 0:1])
            nc.sync.dma_start(out=out[:, S], in_=y3)
        ps2_cm.__exit__(None, None, None)

    nc.finalize()
    return nc


def _get_nc():
    if "nc" not in _CACHE:
        _CACHE["nc"] = _build_nc()
    return _CACHE["nc"]


def _make_in_maps(x, ln_gamma, ln_beta, w_ih, w_hh, b_lstm, w1, b1, w2, b2, w3, b3):
    f32 = np.float32
    x = np.asarray(x, f32)
    ln_gamma = np.asarray(ln_gamma, f32)
    ln_beta = np.asarray(ln_beta, f32)
    w_ih = np.asarray(w_ih, f32)
    wih_f = ln_gamma[:, None] * w_ih                       # (25, 512)
    b_f = np.asarray(b_lstm, f32) + ln_beta @ w_ih         # (512,)
    wa = np.zeros((FP_, G), f32)
    wa[:F] = wih_f
    wa[F] = b_f                                            # ones row (col 25)
    wa[F + 1] = wih_f.sum(0)                               # -mu*r row (col 26)
    wa[:, 256:384] *= 2.0                                  # g-gate sigmoid(2x)
    wa_rep = np.tile(wa, (4, 1))                           # (128, 512)
    whh2 = np.asarray(w_hh, f32).copy()
    whh2[:, 256:384] *= 2.0
    shared = {
        "wa_rep": np.ascontiguousarray(wa_rep).astype(BF16),
        "w_hh": np.ascontiguousarray(whh2).astype(BF16),
        "w1": np.ascontiguousarray(w1, f32).astype(BF16),
        "b1": np.asarray(b1, f32).reshape(H, 1).copy(),
        "w2": np.ascontiguousarray(w2, f32).astype(BF16),
        "b2": np.asarray(b2, f32).reshape(H // 2, 1).copy(),
        "w3": np.ascontiguousarray(w3, f32).astype(BF16),
        "b3": np.asarray(b3, f32).reshape(2, 1).copy(),
    }
    in_maps = []
    for i in range(NCORES):
        xs = x[i * BC:(i + 1) * BC]                        # (BC, T, F)
        x0p = np.zeros((BC, T2, FP_), np.float32)
        x0p[:, :T, :F] = xs
        m = dict(shared)
        m["x0p"] = np.ascontiguousarray(x0p.reshape(BC, T2 * FP_)).astype(BF16)
        in_maps.append(m)
    return in_maps


def _run(in_maps, **kw):
    from concourse.bass_utils import run_bass_kernel_spmd
    nc = _get_nc()
    res = run_bass_kernel_spmd(nc, in_maps, core_ids=list(range(NCORES)), **kw)
    _CACHE["last_results"] = res
    y = np.concatenate([np.asarray(r["out"]).T for r in res.results], axis=0)
    return np.ascontiguousarray(y, np.float32)


def kernel(**inputs):
    return _run(_make_in_maps(**inputs))


# revision 28
# speedup vs baseline: 1.0085x; 1.0085x over previous
"""Trainium2 Bass kernel for nn_BBBLSTM: LayerNorm -> LSTM(25->128, T=30) -> MLP head.

Sharding: data-parallel, batch 8192 -> 1024 per core across 8 NeuronCores.
Weights replicated. No collectives.

v2 design (engine-balanced, latency-pipelined):
  - Phase 0 (batch-major): x staged as [BC, 32t, 32f] bf16 (zero-padded).
    LN stats (sum / sumsq reduce, Square on Act), rstd via sqrt+reciprocal.
    Apply xs = x*rstd in batch-major (free covers all 30 timesteps), the
    -mu*rstd term rides an extra "moving row" (col 26) against a
    column-sum stationary row (rank-1 folding); col 25 = ones for bias.
    Result written to xnd [BC, 1024] DRAM.
  - DMA-transpose (xbar, 16x128 tiles) turns xnd into feature-major
    xt4[g] = [128p = 4t x 32f, 1024 batch] tiles, one per 4-timestep group.
  - Recurrence, K=4 batch chunks of 256 (pipelined chains):
    8 matmuls/chunk -> psum [128, 4*256] (gates i|f|g2|o, g pre-doubled),
    one Sigmoid per chunk covers all four gates (tanh g = 2*sig(2g)-1).
    tanh(c): chunks 0,1 on Act (merged instr), chunks 2,3 as deg-7
    odd polynomial on DVE.  fc = sig_f*c and ig (chunks 0,1) on GPSIMD.
  - MLP head on h_last, output [2, BC] -> host transposes.
"""

import ml_dtypes
import numpy as np

BF16 = ml_dtypes.bfloat16

import concourse.bacc as bacc
import concourse.bass as bass
import concourse.mybir as mybir
from concourse.tile import TileContext

B, T, F, H = 8192, 30, 25, 128
T2 = 32                   # padded timestep slots
FP_ = 32                  # padded feature slots (25 feat + ones + mur + pad)
NCORES = 8
BC = B // NCORES          # 1024 batch rows per core
G = 4 * H                 # 512 gate width
NB = 256                  # batch chunk (4 chunks)
NCHUNK = BC // NB
EPS = 1e-5
FP = mybir.dt.float32
BF = mybir.dt.bfloat16
AF = mybir.ActivationFunctionType
OP = mybir.AluOpType
AX = mybir.AxisListType

# deg-7 odd minimax fit of tanh on [-1.95, 1.95] (maxerr 2.2e-3)
TA0, TA1, TA2, TA3 = 0.9884014, -0.28059538, 0.06143643, -0.00579145

_CACHE = {}

# engine/config knobs (overridable before _build_nc for tuning)
CFG = {
    # per-chunk engine for each elementwise op ("vector" = DVE, "gpsimd" = Pool)
    "tg": ("gpsimd",) * 4,
    "ig": ("gpsimd",) * 4,
    "fc": ("gpsimd",) * 4,
    "add": ("gpsimd",) * 4,
    "h": ("vector",) * 4,
    "poly": ("vector",) * 4,
    "act_tanh_chunks": (0, 1),   # chunks using Act tanh (merged if contiguous)
    "apply_eng": "vector",       # phase-0 LN apply
    "sig_bufs": 3,
    "tmp_bufs": 3,
}


def _bcast_f(ap2d, n):
    """[P, K] AP -> [P, K, n] with stride-0 inner broadcast."""
    return bass.AP(tensor=ap2d.tensor, offset=ap2d.offset,
                   ap=[ap2d.ap[0], ap2d.ap[1], [0, n]])


def _build_nc():
    nc = bacc.Bacc()

    x0p = nc.declare_dram_parameter("x0p", [BC, T2 * FP_], BF, isOutput=False)
    wa_rep = nc.declare_dram_parameter("wa_rep", [128, G], BF, isOutput=False)
    w_hh = nc.declare_dram_parameter("w_hh", [H, G], BF, isOutput=False)
    w1 = nc.declare_dram_parameter("w1", [H, H], BF, isOutput=False)
    b1 = nc.declare_dram_parameter("b1", [H, 1], FP, isOutput=False)
    w2 = nc.declare_dram_parameter("w2", [H, H // 2], BF, isOutput=False)
    b2 = nc.declare_dram_parameter("b2", [H // 2, 1], FP, isOutput=False)
    w3 = nc.declare_dram_parameter("w3", [H // 2, 2], BF, isOutput=False)
    b3 = nc.declare_dram_parameter("b3", [2, 1], FP, isOutput=False)
    out = nc.declare_dram_parameter("out", [2, BC], FP, isOutput=True)

    xnd = nc.dram_tensor("xnd", [BC, T2 * FP_], BF)

    from contextlib import ExitStack

    with TileContext(nc) as tc, ExitStack() as ctx:
        consts = ctx.enter_context(tc.tile_pool(name="consts", bufs=1))
        state = ctx.enter_context(tc.tile_pool(name="state", bufs=1))
        p0 = ctx.enter_context(tc.tile_pool(name="p0", bufs=3))
        p0s = ctx.enter_context(tc.tile_pool(name="p0s", bufs=6))
        sigp = ctx.enter_context(tc.tile_pool(name="sigp", bufs=CFG["sig_bufs"]))
        tmpp = ctx.enter_context(tc.tile_pool(name="tmpp", bufs=CFG["tmp_bufs"]))
        mlpp = ctx.enter_context(tc.tile_pool(name="mlpp", bufs=2))

        # ---- constants ----
        wa_sb = consts.tile([128, G], BF)
        nc.sync.dma_start(out=wa_sb, in_=wa_rep[:, :])
        whh_sb = consts.tile([H, G], BF)
        nc.gpsimd.dma_start(out=whh_sb, in_=w_hh[:, :])
        w1_sb = consts.tile([H, H], BF)
        nc.gpsimd.dma_start(out=w1_sb, in_=w1[:, :])
        b1_sb = consts.tile([H, 1], FP)
        nc.sync.dma_start(out=b1_sb, in_=b1[:, :])
        w2_sb = consts.tile([H, H // 2], BF)
        nc.gpsimd.dma_start(out=w2_sb, in_=w2[:, :])
        b2_sb = consts.tile([H // 2, 1], FP)
        nc.sync.dma_start(out=b2_sb, in_=b2[:, :])
        w3_sb = consts.tile([H // 2, 2], BF)
        nc.gpsimd.dma_start(out=w3_sb, in_=w3[:, :])
        b3_sb = consts.tile([2, 1], FP)
        nc.sync.dma_start(out=b3_sb, in_=b3[:, :])
        eps_sb = consts.tile([128, 1], FP)
        nc.vector.memset(eps_sb, EPS)

        # ---- phase 0: LN stats + apply, batch-major, 8 tiles of 128 rows ----
        NG = T2 // 4
        xnd_m = xnd[:, :].rearrange("b (g c) -> b g c", c=128)
        xt4 = [state.tile([128, BC], BF, name=f"xt4_{g}") for g in range(NG)]

        NT = BC // 128
        for i in range(NT):
            rows = slice(i * 128, (i + 1) * 128)
            x0 = p0.tile([128, T2 * FP_], BF, tag="x0")
            nc.gpsimd.dma_start(out=x0, in_=x0p[rows, :])
            x0v = x0[:, :].rearrange("p (t f) -> p t f", f=FP_)

            # stats reduces on unpadded [T2, F] strided views (free 800 not 1024)
            s1 = p0s.tile([128, T2], FP, tag="s1")
            nc.vector.tensor_reduce(out=s1, in_=x0v[:, :, 0:F], axis=AX.X,
                                    op=OP.add)
            x2 = p0.tile([128, T2 * FP_], BF, tag="x2")
            x2v = x2[:, :].rearrange("p (t f) -> p t f", f=FP_)
            nc.scalar.activation(x2v[:, :, 0:F], x0v[:, :, 0:F], AF.Square)
            s2 = p0s.tile([128, T2], FP, tag="s2")
            nc.vector.tensor_reduce(out=s2, in_=x2v[:, :, 0:F], axis=AX.X,
                                    op=OP.add)

            ve = nc.vector
            mu_n = p0s.tile([128, T2], BF, tag="mu_n")
            ve.tensor_scalar_mul(mu_n, s1, -1.0 / F)
            mu2 = p0s.tile([128, T2], FP, tag="mu2")
            ve.tensor_mul(mu2, mu_n, mu_n)
            mu2e = p0s.tile([128, T2], FP, tag="mu2e")
            ve.tensor_scalar_sub(mu2e, mu2, EPS)
            w0 = p0s.tile([128, T2], FP, tag="w0")
            ve.scalar_tensor_tensor(
                out=w0, in0=s2, scalar=1.0 / F, in1=mu2e,
                op0=OP.mult, op1=OP.subtract)
            if True:
                # Act Sqrt + exact reciprocal
                sd = p0s.tile([128, T2], FP, tag="sd")
                nc.scalar.activation(sd, w0, AF.Sqrt)
                r_ = p0s.tile([128, T2], BF, tag="r")
                with nc.allow_low_precision(reason="bf16 rstd for LN"):
                    nc.vector.reciprocal(r_, sd)
            elif False:
                # overlapped tiles: rsqrt = poly seed + 2 Newton iterations
                # on DVE so no act-table switch mid-recurrence
                RC0, RC1, RC2 = 2.36276903, -1.63315947, 0.37159572
                t0_ = p0s.tile([128, T2], FP, tag="t0")
                ve.tensor_scalar(out=t0_, in0=w0, scalar1=RC2, scalar2=RC1,
                                 op0=OP.mult, op1=OP.add)
                y0 = p0s.tile([128, T2], FP, tag="y0")
                ve.tensor_mul(y0, t0_, w0)
                yc = p0s.tile([128, T2], FP, tag="yc")
                ve.tensor_scalar_add(yc, y0, RC0)
                for it in range(2):
                    t1 = p0s.tile([128, T2], FP, tag=f"nt1_{it}")
                    ve.tensor_mul(t1, yc, yc)
                    t2 = p0s.tile([128, T2], FP, tag=f"nt2_{it}")
                    ve.tensor_mul(t2, w0, t1)
                    t3 = p0s.tile([128, T2], FP, tag=f"nt3_{it}")
                    ve.tensor_scalar(out=t3, in0=t2, scalar1=-0.5,
                                     scalar2=1.5, op0=OP.mult, op1=OP.add)
                    ycn = p0s.tile([128, T2], FP if it == 0 else BF,
                                   tag=f"ycn_{it}")
                    ve.tensor_mul(ycn, yc, t3)
                    yc = ycn
                r_ = yc

            xn = p0.tile([128, T2 * FP_], BF, tag="xn")
            xnv = xn[:, :].rearrange("p (t f) -> p t f", f=FP_)
            getattr(nc, CFG["apply_eng"]).tensor_mul(
                xnv, x0v, _bcast_f(r_[:, :], FP_))
            # col 26 = -mu*rstd (rank-1 row against column-sum stationary)
            nc.gpsimd.tensor_mul(xnv[:, :, 26:27], mu_n, r_)
            # col 25 = ones (bias row)
            nc.vector.memset(xnv[:, :, 25:26], 1.0)
            nc.sync.dma_start(out=xnd[rows, :], in_=xn)

            # transposes per batch-half: the left half (tiles 0-3) unblocks
            # chunks 0,1 of the recurrence while tiles 4-7 finish phase 0
            if i in (3, 7):
                half = i // 4
                hrows = slice(half * 512, (half + 1) * 512)
                for g in range(NG):
                    nc.sync.dma_start_transpose(out=xt4[g][:, hrows],
                                                in_=xnd_m[hrows, g, :])

        # ---- persistent LSTM state ----
        h = state.tile([H, BC], BF)
        c = state.tile([H, BC], BF)
        nc.vector.memset(h, 0.0)
        nc.vector.memset(c, 0.0)

        # gate column slices in psum: i | f | g2 | o
        SI, SF, SG, SO = (slice(k * NB, (k + 1) * NB) for k in range(4))

        ps_cm = tc.tile_pool(name="ps", bufs=1, space="PSUM")
        psp = ps_cm.__enter__()
        for t in range(T):
            g, rr = divmod(t, 4)
            pb = 32 * rr
            sigs = []
            for cc in range(NCHUNK):
                S = slice(cc * NB, (cc + 1) * NB)
                ps = psp.tile([128, 4 * NB], FP, tag=f"ps{cc}")
                for k, gsl in enumerate(
                        (slice(0, 128), slice(128, 256),
                         slice(256, 384), slice(384, 512))):
                    d = ps[:, k * NB:(k + 1) * NB]
                    nc.tensor.matmul(d, wa_sb[pb:pb + 32, gsl],
                                     xt4[g][pb:pb + 32, S],
                                     start=True, stop=False,
                                     tile_position=(pb, 0))
                    nc.tensor.matmul(d, whh_sb[:, gsl], h[:, S],
                                     start=False, stop=True)
                sig = sigp.tile([128, 4 * NB], BF, tag=f"sig{cc}")
                nc.scalar.activation(sig, ps, AF.Sigmoid)
                sigs.append(sig)

                E = lambda op: getattr(nc, CFG[op][cc])
                tg = tmpp.tile([128, NB], BF, tag=f"tg{cc}")
                E("tg").tensor_scalar(
                    out=tg, in0=sig[:, SG], scalar1=2.0, scalar2=-1.0,
                    op0=OP.mult, op1=OP.add)
                ig = tmpp.tile([128, NB], BF, tag=f"ig{cc}")
                E("ig").tensor_mul(ig, sig[:, SI], tg)
                fc = tmpp.tile([128, NB], BF, tag=f"fc{cc}")
                E("fc").tensor_mul(fc, sig[:, SF], c[:, S])
                E("add").tensor_add(c[:, S], fc, ig)

                if cc not in CFG["act_tanh_chunks"]:
                    # tanh(c) deg-7 odd polynomial
                    pe_ = getattr(nc, CFG["poly"][cc])
                    u = tmpp.tile([128, NB], BF, tag=f"u{cc}")
                    pe_.tensor_mul(u, c[:, S], c[:, S])
                    v = tmpp.tile([128, NB], BF, tag=f"v{cc}")
                    pe_.tensor_scalar(out=v, in0=u, scalar1=TA3,
                                      scalar2=TA2, op0=OP.mult, op1=OP.add)
                    w_ = tmpp.tile([128, NB], BF, tag=f"w{cc}")
                    pe_.tensor_mul(w_, v, u)
                    z = tmpp.tile([128, NB], BF, tag=f"z{cc}")
                    pe_.tensor_scalar_add(z, w_, TA1)
                    zz = tmpp.tile([128, NB], BF, tag=f"zz{cc}")
                    pe_.tensor_mul(zz, z, u)
                    p_ = tmpp.tile([128, NB], BF, tag=f"p{cc}")
                    pe_.tensor_scalar_add(p_, zz, TA0)
                    tc_ = tmpp.tile([128, NB], BF, tag=f"tc{cc}")
                    pe_.tensor_mul(tc_, p_, c[:, S])
                    E("h").tensor_mul(h[:, S], sigs[cc][:, SO], tc_)

            # Act-tanh chunks: exact tanh, merged instr if contiguous
            atc = CFG["act_tanh_chunks"]
            if atc:
                lo, hi = min(atc), max(atc) + 1
                tc01 = tmpp.tile([128, (hi - lo) * NB], BF, tag="tc01")
                nc.scalar.activation(tc01, c[:, lo * NB:hi * NB], AF.Tanh)
                for cc in atc:
                    S = slice(cc * NB, (cc + 1) * NB)
                    getattr(nc, CFG["h"][cc]).tensor_mul(
                        h[:, S], sigs[cc][:, SO],
                        tc01[:, (cc - lo) * NB:(cc - lo + 1) * NB])

        ps_cm.__exit__(None, None, None)

        # ---- MLP head ----
        ps2_cm = tc.tile_pool(name="ps2", bufs=2, space="PSUM")
        ps2 = ps2_cm.__enter__()
        for cc in range(2):
            S = slice(cc * 512, (cc + 1) * 512)
            m1 = ps2.tile([H, 512], FP, tag="m1")
            nc.tensor.matmul(m1, w1_sb, h[:, S], start=True, stop=True)
            y1 = mlpp.tile([H, 512], BF, tag="y1")
            nc.scalar.activation(y1, m1, AF.Relu, bias=b1_sb[:, 0:1])
            m2 = ps2.tile([H // 2, 512], FP, tag="m2")
            nc.tensor.matmul(m2, w2_sb, y1, start=True, stop=True)
            y2 = mlpp.tile([H // 2, 512], BF, tag="y2")
            nc.scalar.activation(y2, m2, AF.Relu, bias=b2_sb[:, 0:1])
            m3 = ps2.tile([2, 512], FP, tag="m3")
            nc.tensor.matmul(m3, w3_sb, y2, start=True, stop=True)
            y3 = mlpp.tile([2, 512], FP, tag="y3")
            nc.vector.tensor_scalar_add(y3, m3, b3_sb[:, 0:1])
            nc.sync.dma_start(out=out[:, S], in_=y3)
        ps2_cm.__exit__(None, None, None)

    nc.finalize()
    return nc


def _get_nc():
    if "nc" not in _CACHE:
        _CACHE["nc"] = _build_nc()
    return _CACHE["nc"]


def _make_in_maps(x, ln_gamma, ln_beta, w_ih, w_hh, b_lstm, w1, b1, w2, b2, w3, b3):
    f32 = np.float32
    x = np.asarray(x, f32)
    ln_gamma = np.asarray(ln_gamma, f32)
    ln_beta = np.asarray(ln_beta, f32)
    w_ih = np.asarray(w_ih, f32)
    wih_f = ln_gamma[:, None] * w_ih                       # (25, 512)
    b_f = np.asarray(b_lstm, f32) + ln_beta @ w_ih         # (512,)
    wa = np.zeros((FP_, G), f32)
    wa[:F] = wih_f
    wa[F] = b_f                                            # ones row (col 25)
    wa[F + 1] = wih_f.sum(0)                               # -mu*r row (col 26)
    wa[:, 256:384] *= 2.0                                  # g-gate sigmoid(2x)
    wa_rep = np.tile(wa, (4, 1))                           # (128, 512)
    whh2 = np.asarray(w_hh, f32).copy()
    whh2[:, 256:384] *= 2.0
    shared = {
        "wa_rep": np.ascontiguousarray(wa_rep).astype(BF16),
        "w_hh": np.ascontiguousarray(whh2).astype(BF16),
        "w1": np.ascontiguousarray(w1, f32).astype(BF16),
        "b1": np.asarray(b1, f32).reshape(H, 1).copy(),
        "w2": np.ascontiguousarray(w2, f32).astype(BF16),
        "b2": np.asarray(b2, f32).reshape(H // 2, 1).copy(),
        "w3": np.ascontiguousarray(w3, f32).astype(BF16),
        "b3": np.asarray(b3, f32).reshape(2, 1).copy(),
    }
    in_maps = []
    for i in range(NCORES):
        xs = x[i * BC:(i + 1) * BC]                        # (BC, T, F)
        x0p = np.zeros((BC, T2, FP_), np.float32)
        x0p[:, :T, :F] = xs
        m = dict(shared)
        m["x0p"] = np.ascontiguousarray(x0p.reshape(BC, T2 * FP_)).astype(BF16)
        in_maps.append(m)
    return in_maps


def _run(in_maps, **kw):
    from concourse.bass_utils import run_bass_kernel_spmd
    nc = _get_nc()
    res = run_bass_kernel_spmd(nc, in_maps, core_ids=list(range(NCORES)), **kw)
    _CACHE["last_results"] = res
    y = np.concatenate([np.asarray(r["out"]).T for r in res.results], axis=0)
    return np.ascontiguousarray(y, np.float32)


def kernel(**inputs):
    return _run(_make_in_maps(**inputs))


# revision 29
# speedup vs baseline: 1.1590x; 1.1493x over previous
"""Trainium2 Bass kernel for nn_BBBLSTM: LayerNorm -> LSTM(25->128, T=30) -> MLP head.

Sharding: data-parallel, batch 8192 -> 1024 per core across 8 NeuronCores.
Weights replicated. No collectives.

v2 design (engine-balanced, latency-pipelined):
  - Phase 0 (batch-major): x staged as [BC, 32t, 32f] bf16 (zero-padded).
    LN stats (sum / sumsq reduce, Square on Act), rstd via sqrt+reciprocal.
    Apply xs = x*rstd in batch-major (free covers all 30 timesteps), the
    -mu*rstd term rides an extra "moving row" (col 26) against a
    column-sum stationary row (rank-1 folding); col 25 = ones for bias.
    Result written to xnd [BC, 1024] DRAM.
  - DMA-transpose (xbar, 16x128 tiles) turns xnd into feature-major
    xt4[g] = [128p = 4t x 32f, 1024 batch] tiles, one per 4-timestep group.
  - Recurrence, K=4 batch chunks of 256 (pipelined chains):
    8 matmuls/chunk -> psum [128, 4*256] (gates i|f|g2|o, g pre-doubled),
    one Sigmoid per chunk covers all four gates (tanh g = 2*sig(2g)-1).
    tanh(c): chunks 0,1 on Act (merged instr), chunks 2,3 as deg-7
    odd polynomial on DVE.  fc = sig_f*c and ig (chunks 0,1) on GPSIMD.
  - MLP head on h_last, output [2, BC] -> host transposes.
"""

import ml_dtypes
import numpy as np

BF16 = ml_dtypes.bfloat16

import concourse.bacc as bacc
import concourse.bass as bass
import concourse.mybir as mybir
from concourse.tile import TileContext

B, T, F, H = 8192, 30, 25, 128
T2 = 32                   # padded timestep slots
FP_ = 32                  # padded feature slots (25 feat + ones + mur + pad)
NCORES = 8
BC = B // NCORES          # 1024 batch rows per core
G = 4 * H                 # 512 gate width
NB = 256                  # batch chunk (4 chunks)
NCHUNK = BC // NB
EPS = 1e-5
FP = mybir.dt.float32
BF = mybir.dt.bfloat16
AF = mybir.ActivationFunctionType
OP = mybir.AluOpType
AX = mybir.AxisListType

# deg-7 odd minimax fit of tanh on [-1.95, 1.95] (maxerr 2.2e-3)
TA0, TA1, TA2, TA3 = 0.9884014, -0.28059538, 0.06143643, -0.00579145

_CACHE = {}

# engine/config knobs (overridable before _build_nc for tuning)
CFG = {
    # per-chunk engine for each elementwise op ("vector" = DVE, "gpsimd" = Pool)
    "tg": ("gpsimd",) * 4,
    "ig": ("gpsimd",) * 4,
    "fc": ("gpsimd",) * 4,
    "add": ("gpsimd",) * 4,
    "h": ("vector",) * 4,
    "poly": ("vector",) * 4,
    "act_tanh_chunks": (0, 1),   # chunks using Act tanh (merged if contiguous)
    "apply_eng": "vector",       # phase-0 LN apply
    "sig_bufs": 3,
    "tmp_bufs": 3,
}


def _bcast_f(ap2d, n):
    """[P, K] AP -> [P, K, n] with stride-0 inner broadcast."""
    return bass.AP(tensor=ap2d.tensor, offset=ap2d.offset,
                   ap=[ap2d.ap[0], ap2d.ap[1], [0, n]])


def _build_nc():
    nc = bacc.Bacc()

    x0p = nc.declare_dram_parameter("x0p", [BC, T2 * FP_], BF, isOutput=False)
    wa_rep = nc.declare_dram_parameter("wa_rep", [128, G], BF, isOutput=False)
    w_hh = nc.declare_dram_parameter("w_hh", [H, G], BF, isOutput=False)
    w1 = nc.declare_dram_parameter("w1", [H, H], BF, isOutput=False)
    b1 = nc.declare_dram_parameter("b1", [H, 1], FP, isOutput=False)
    w2 = nc.declare_dram_parameter("w2", [H, H // 2], BF, isOutput=False)
    b2 = nc.declare_dram_parameter("b2", [H // 2, 1], FP, isOutput=False)
    w3 = nc.declare_dram_parameter("w3", [H // 2, 2], BF, isOutput=False)
    b3 = nc.declare_dram_parameter("b3", [2, 1], FP, isOutput=False)
    out = nc.declare_dram_parameter("out", [2, BC], FP, isOutput=True)

    xnd = nc.dram_tensor("xnd", [BC, T2 * FP_], BF)

    from contextlib import ExitStack

    with TileContext(nc) as tc, ExitStack() as ctx:
        consts = ctx.enter_context(tc.tile_pool(name="consts", bufs=1))
        state = ctx.enter_context(tc.tile_pool(name="state", bufs=1))
        p0 = ctx.enter_context(tc.tile_pool(name="p0", bufs=3))
        p0s = ctx.enter_context(tc.tile_pool(name="p0s", bufs=6))
        sigp = ctx.enter_context(tc.tile_pool(name="sigp", bufs=CFG["sig_bufs"]))
        tmpp = ctx.enter_context(tc.tile_pool(name="tmpp", bufs=CFG["tmp_bufs"]))
        mlpp = ctx.enter_context(tc.tile_pool(name="mlpp", bufs=2))

        # ---- constants ----
        wa_sb = consts.tile([128, G], BF)
        nc.sync.dma_start(out=wa_sb, in_=wa_rep[:, :])
        whh_sb = consts.tile([H, G], BF)
        nc.gpsimd.dma_start(out=whh_sb, in_=w_hh[:, :])
        w1_sb = consts.tile([H, H], BF)
        nc.gpsimd.dma_start(out=w1_sb, in_=w1[:, :])
        b1_sb = consts.tile([H, 1], FP)
        nc.sync.dma_start(out=b1_sb, in_=b1[:, :])
        w2_sb = consts.tile([H, H // 2], BF)
        nc.gpsimd.dma_start(out=w2_sb, in_=w2[:, :])
        b2_sb = consts.tile([H // 2, 1], FP)
        nc.sync.dma_start(out=b2_sb, in_=b2[:, :])
        w3_sb = consts.tile([H // 2, 2], BF)
        nc.gpsimd.dma_start(out=w3_sb, in_=w3[:, :])
        b3_sb = consts.tile([2, 1], FP)
        nc.sync.dma_start(out=b3_sb, in_=b3[:, :])
        eps_sb = consts.tile([128, 1], FP)
        nc.vector.memset(eps_sb, EPS)

        # ---- phase 0: LN stats + apply, batch-major, 8 tiles of 128 rows ----
        NG = T2 // 4
        xnd_m = xnd[:, :].rearrange("b (g c) -> b g c", c=128)
        xt4 = [state.tile([128, BC], BF, name=f"xt4_{g}") for g in range(NG)]

        NT = BC // 128
        for i in range(NT):
            rows = slice(i * 128, (i + 1) * 128)
            x0 = p0.tile([128, T2 * FP_], BF, tag="x0")
            nc.gpsimd.dma_start(out=x0, in_=x0p[rows, :])
            x0v = x0[:, :].rearrange("p (t f) -> p t f", f=FP_)

            # stats reduces on unpadded [T2, F] strided views (free 800 not 1024)
            s1 = p0s.tile([128, T2], FP, tag="s1")
            nc.vector.tensor_reduce(out=s1, in_=x0v[:, :, 0:F], axis=AX.X,
                                    op=OP.add)
            x2 = p0.tile([128, T2 * FP_], BF, tag="x2")
            x2v = x2[:, :].rearrange("p (t f) -> p t f", f=FP_)
            nc.scalar.activation(x2v[:, :, 0:F], x0v[:, :, 0:F], AF.Square)
            s2 = p0s.tile([128, T2], FP, tag="s2")
            nc.vector.tensor_reduce(out=s2, in_=x2v[:, :, 0:F], axis=AX.X,
                                    op=OP.add)

            mu_n = p0s.tile([128, T2], BF, tag="mu_n")
            nc.vector.tensor_scalar_mul(mu_n, s1, -1.0 / F)
            mu2 = p0s.tile([128, T2], FP, tag="mu2")
            nc.gpsimd.tensor_mul(mu2, mu_n, mu_n)
            var = p0s.tile([128, T2], FP, tag="var")
            nc.vector.scalar_tensor_tensor(
                out=var, in0=s2, scalar=1.0 / F, in1=mu2,
                op0=OP.mult, op1=OP.subtract)
            sd = p0s.tile([128, T2], FP, tag="sd")
            nc.scalar.activation(sd, var, AF.Sqrt, bias=eps_sb[:, 0:1])
            r_ = p0s.tile([128, T2], BF, tag="r")
            with nc.allow_low_precision(reason="bf16 rstd is enough for LN"):
                nc.vector.reciprocal(r_, sd)

            xn = p0.tile([128, T2 * FP_], BF, tag="xn")
            xnv = xn[:, :].rearrange("p (t f) -> p t f", f=FP_)
            getattr(nc, CFG["apply_eng"]).tensor_mul(
                xnv, x0v, _bcast_f(r_[:, :], FP_))
            # col 26 = -mu*rstd (rank-1 row against column-sum stationary)
            nc.gpsimd.tensor_mul(xnv[:, :, 26:27], mu_n, r_)
            # col 25 = ones (bias row)
            nc.vector.memset(xnv[:, :, 25:26], 1.0)
            nc.sync.dma_start(out=xnd[rows, :], in_=xn)

            # transposes per batch-half: the left half (tiles 0-3) unblocks
            # chunks 0,1 of the recurrence while tiles 4-7 finish phase 0
            if i in (3, 7):
                half = i // 4
                hrows = slice(half * 512, (half + 1) * 512)
                for g in range(NG):
                    nc.sync.dma_start_transpose(out=xt4[g][:, hrows],
                                                in_=xnd_m[hrows, g, :])

        # ---- persistent LSTM state ----
        h = state.tile([H, BC], BF)
        c = state.tile([H, BC], BF)
        nc.vector.memset(h, 0.0)
        nc.vector.memset(c, 0.0)

        # gate column slices in psum: i | f | g2 | o
        SI, SF, SG, SO = (slice(k * NB, (k + 1) * NB) for k in range(4))

        ps_cm = tc.tile_pool(name="ps", bufs=1, space="PSUM")
        psp = ps_cm.__enter__()
        for t in range(T):
            g, rr = divmod(t, 4)
            pb = 32 * rr
            sigs = []
            for cc in range(NCHUNK):
                S = slice(cc * NB, (cc + 1) * NB)
                ps = psp.tile([128, 4 * NB], FP, tag=f"ps{cc}")
                for k, gsl in enumerate(
                        (slice(0, 128), slice(128, 256),
                         slice(256, 384), slice(384, 512))):
                    d = ps[:, k * NB:(k + 1) * NB]
                    nc.tensor.matmul(d, wa_sb[pb:pb + 32, gsl],
                                     xt4[g][pb:pb + 32, S],
                                     start=True, stop=False,
                                     tile_position=(pb, 0))
                    nc.tensor.matmul(d, whh_sb[:, gsl], h[:, S],
                                     start=False, stop=True)
                sig = sigp.tile([128, 4 * NB], BF, tag=f"sig{cc}")
                nc.scalar.activation(sig, ps, AF.Sigmoid)
                sigs.append(sig)

                E = lambda op: getattr(nc, CFG[op][cc])
                tg = tmpp.tile([128, NB], BF, tag=f"tg{cc}")
                E("tg").tensor_scalar(
                    out=tg, in0=sig[:, SG], scalar1=2.0, scalar2=-1.0,
                    op0=OP.mult, op1=OP.add)
                ig = tmpp.tile([128, NB], BF, tag=f"ig{cc}")
                E("ig").tensor_mul(ig, sig[:, SI], tg)
                fc = tmpp.tile([128, NB], BF, tag=f"fc{cc}")
                E("fc").tensor_mul(fc, sig[:, SF], c[:, S])
                E("add").tensor_add(c[:, S], fc, ig)

                if cc not in CFG["act_tanh_chunks"]:
                    # tanh(c) deg-7 odd polynomial
                    pe_ = getattr(nc, CFG["poly"][cc])
                    u = tmpp.tile([128, NB], BF, tag=f"u{cc}")
                    pe_.tensor_mul(u, c[:, S], c[:, S])
                    v = tmpp.tile([128, NB], BF, tag=f"v{cc}")
                    pe_.tensor_scalar(out=v, in0=u, scalar1=TA3,
                                      scalar2=TA2, op0=OP.mult, op1=OP.add)
                    w_ = tmpp.tile([128, NB], BF, tag=f"w{cc}")
                    pe_.tensor_mul(w_, v, u)
                    z = tmpp.tile([128, NB], BF, tag=f"z{cc}")
                    pe_.tensor_scalar_add(z, w_, TA1)
                    zz = tmpp.tile([128, NB], BF, tag=f"zz{cc}")
                    pe_.tensor_mul(zz, z, u)
                    p_ = tmpp.tile([128, NB], BF, tag=f"p{cc}")
                    pe_.tensor_scalar_add(p_, zz, TA0)
                    tc_ = tmpp.tile([128, NB], BF, tag=f"tc{cc}")
                    pe_.tensor_mul(tc_, p_, c[:, S])
                    E("h").tensor_mul(h[:, S], sigs[cc][:, SO], tc_)

            # Act-tanh chunks: exact tanh, merged instr if contiguous
            atc = CFG["act_tanh_chunks"]
            if atc:
                lo, hi = min(atc), max(atc) + 1
                tc01 = tmpp.tile([128, (hi - lo) * NB], BF, tag="tc01")
                nc.scalar.activation(tc01, c[:, lo * NB:hi * NB], AF.Tanh)
                for cc in atc:
                    S = slice(cc * NB, (cc + 1) * NB)
                    getattr(nc, CFG["h"][cc]).tensor_mul(
                        h[:, S], sigs[cc][:, SO],
                        tc01[:, (cc - lo) * NB:(cc - lo + 1) * NB])

        ps_cm.__exit__(None, None, None)

        # ---- MLP head ----
        ps2_cm = tc.tile_pool(name="ps2", bufs=2, space="PSUM")
        ps2 = ps2_cm.__enter__()
        for cc in range(2):
            S = slice(cc * 512, (cc + 1) * 512)
            m1 = ps2.tile([H, 512], FP, tag="m1")
            nc.tensor.matmul(m1, w1_sb, h[:, S], start=True, stop=True)
            y1 = mlpp.tile([H, 512], BF, tag="y1")
            nc.scalar.activation(y1, m1, AF.Relu, bias=b1_sb[:, 0:1])
            m2 = ps2.tile([H // 2, 512], FP, tag="m2")
            nc.tensor.matmul(m2, w2_sb, y1, start=True, stop=True)
            y2 = mlpp.tile([H // 2, 512], BF, tag="y2")
            nc.scalar.activation(y2, m2, AF.Relu, bias=b2_sb[:, 0:1])
            m3 = ps2.tile([2, 512], FP, tag="m3")
            nc.tensor.matmul(m3, w3_sb, y2, start=True, stop=True)
            y3 = mlpp.tile([2, 512], FP, tag="y3")
            nc.vector.tensor_scalar_add(y3, m3, b3_sb[:, 0:1])
            nc.sync.dma_start(out=out[:, S], in_=y3)
        ps2_cm.__exit__(None, None, None)

    nc.finalize()
    return nc


def _get_nc():
    if "nc" not in _CACHE:
        _CACHE["nc"] = _build_nc()
    return _CACHE["nc"]


def _make_in_maps(x, ln_gamma, ln_beta, w_ih, w_hh, b_lstm, w1, b1, w2, b2, w3, b3):
    f32 = np.float32
    x = np.asarray(x, f32)
    ln_gamma = np.asarray(ln_gamma, f32)
    ln_beta = np.asarray(ln_beta, f32)
    w_ih = np.asarray(w_ih, f32)
    wih_f = ln_gamma[:, None] * w_ih                       # (25, 512)
    b_f = np.asarray(b_lstm, f32) + ln_beta @ w_ih         # (512,)
    wa = np.zeros((FP_, G), f32)
    wa[:F] = wih_f
    wa[F] = b_f                                            # ones row (col 25)
    wa[F + 1] = wih_f.sum(0)                               # -mu*r row (col 26)
    wa[:, 256:384] *= 2.0                                  # g-gate sigmoid(2x)
    wa_rep = np.tile(wa, (4, 1))                           # (128, 512)
    whh2 = np.asarray(w_hh, f32).copy()
    whh2[:, 256:384] *= 2.0
    shared = {
        "wa_rep": np.ascontiguousarray(wa_rep).astype(BF16),
        "w_hh": np.ascontiguousarray(whh2).astype(BF16),
        "w1": np.ascontiguousarray(w1, f32).astype(BF16),
        "b1": np.asarray(b1, f32).reshape(H, 1).copy(),
        "w2": np.ascontiguousarray(w2, f32).astype(BF16),
        "b2": np.asarray(b2, f32).reshape(H // 2, 1).copy(),
        "w3": np.ascontiguousarray(w3, f32).astype(BF16),
        "b3": np.asarray(b3, f32).reshape(2, 1).copy(),
    }
    in_maps = []
    for i in range(NCORES):
        xs = x[i * BC:(i + 1) * BC]                        # (BC, T, F)
        x0p = np.zeros((BC, T2, FP_), np.float32)
        x0p[:, :T, :F] = xs
        m = dict(shared)
        m["x0p"] = np.ascontiguousarray(x0p.reshape(BC, T2 * FP_)).astype(BF16)
        in_maps.append(m)
    return in_maps


def _run(in_maps, **kw):
    from concourse.bass_utils import run_bass_kernel_spmd
    nc = _get_nc()
    res = run_bass_kernel_spmd(nc, in_maps, core_ids=list(range(NCORES)), **kw)
    _CACHE["last_results"] = res
    y = np.concatenate([np.asarray(r["out"]).T for r in res.results], axis=0)
    return np.ascontiguousarray(y, np.float32)


def kernel(**inputs):
    return _run(_make_in_maps(**inputs))
